# revision 24
# baseline (speedup 1.0000x reference)
"""CrossHeadAttention Trainium2 kernel (8-core SPMD, data+head parallel).

Reference computation (per batch b):
    k = x_enc @ Wk ; v = x_enc @ Wv ; q = x @ Wq        (bias-free linears)
    wei = softmax((q @ k^T) / sqrt(1024))  per head
    out = wei @ v                                        -> [B, T, H, D]

Sharding: 8 cores = 2 batches x 4 head-groups (4 heads each). Each core
receives x[b], x_enc[b] (host-cast to bf16) and the 256-column slice of
Wq/Wk/Wv for its heads, and produces the unnormalized attention numerator
plus the softmax denominator; the host divides and gathers.

Per-core dataflow (all matmuls bf16 = 1 col/cycle on the PE):
  x, x_enc --HW DMA-transpose (bf16, 16x128 xbar tiles)--> xT/xeT in SBUF
    (zero PE cost; the PE never transposes activations)
  qT/kT[d,t] = W-stationary matmuls; psum->sbuf bf16 rounding on DVE
  vT chunks --DMA-transpose--> v_sb[s, st, head, d] (+ones column at d=64
    so the PV matmul also produces the softmax denominator)
  S^T[s,t] = kT^T qT per head (K=64, two heads row-packed in partitions)
  p = exp(S/32): even s-tiles exact on ScalarE (act table, psum->sbuf
    bf16), odd s-tiles on DVE via a calibrated Schraudolph bit-trick:
    bf16 bits of exp2(x) ~ int16(round(128*log2(e)*x/32 + 16250)); the
    constant-factor part of its bias cancels in the softmax ratio,
    leaving ~1% noise on half the weights (measured end-to-end err
    ~8.4e-3 vs the 2e-2 gate)
  num^T[t, d+1] = p-stationary PV matmuls: out[t-block, 65] accumulated
    over s-tiles into a single psum bank (DVE pre-memset + start=False
    so four 65-wide accumulators share one bank without zero-region
    clobber); col 64 = denominator. DVE copies psum->sbuf, SWDGE stores.

Engine budget per core (TimelineSim): PE ~296k cycles (123us) = proj 98k
+ scores 131k + PV 67k; ACT ~85us exp; DVE ~85us (schrau + copies); DMA
~45us. The kernel is PE-bound; exp hides under the matmul stream.
"""

from contextlib import ExitStack

import numpy as np
import ml_dtypes

import concourse.bacc as bacc
import concourse.tile as tile
from concourse import mybir
from concourse.bass_utils import run_bass_kernel_spmd

# Problem constants (hardcoded per spec)
B = 2
T = 2048          # query length
S = 2048          # key/value length
C = 1024          # n_embd
H = 16            # total heads
D = 64            # head size
N_CORES = 8
HG = H // (N_CORES // B)       # heads per core = 4
DCORE = HG * D                 # 256 projected dims per core
P = 128                        # partitions
CT = C // P                    # 8 contraction tiles
NPAIR = HG // 2                # 2 head pairs per core
TCH = 512                      # t-chunk width in attention
NTCH = T // TCH                # 4
ST = S // P                    # 16 s-tiles
NCH = 4                        # 512-row input chunks

F32 = mybir.dt.float32
BF16 = mybir.dt.bfloat16
I16 = mybir.dt.int16
AF = mybir.ActivationFunctionType

SCALE = float(C) ** -0.5       # 1/32, folded into the exp activation

# Schraudolph exp for the DVE share: bf16 bitpattern of exp(s*SCALE) ~
# round(A*s + B); B = 16256 (bf16 exponent bias<<7) + m, m=-6 calibrated
# end-to-end for round-to-nearest int16 conversion.
SCH_A = 128.0 * float(np.log2(np.e)) * SCALE
SCH_B = 16256.0 - 6.0
LOOK = 2                       # score-emission lookahead (units)


def _build_body(nc, tc, x, xe, wq, wk, wv, o):
    with ExitStack() as ctx:
        big = ctx.enter_context(tc.tile_pool(name="big", bufs=1))

        # per-chunk contiguous tiles: the DMA xbar transpose requires a
        # contiguous output access pattern
        xT = [big.tile([P, CT, TCH], BF16, tag=f"xT{c}", name=f"xT{c}")
              for c in range(NCH)]
        xeT = [big.tile([P, CT, TCH], BF16, tag=f"xeT{c}", name=f"xeT{c}")
               for c in range(NCH)]
        kT = big.tile([P, NPAIR, S], BF16, tag="kT")
        qT = big.tile([P, NPAIR, T], BF16, tag="qT")
        # v, with a ones column appended per head (col D) for softmax sums
        v_sb = big.tile([P, ST, HG, D + 1], BF16, tag="v_sb")
        nc.vector.memset(v_sb[:, :, :, D], 1.0)

        # prime the ScalarE exp table at t=0 so the table load is off the
        # critical path of the first real exp
        dummy = big.tile([1, 2], F32, tag="dummy")
        nc.vector.memset(dummy, 0.0)
        nc.scalar.activation(out=dummy, in_=dummy, func=AF.Exp)

        # DMA order on the serial xbar/DMA-engine resource: first x chunk,
        # then Wq (unblocks qT0), first xe chunk, Wk/Wv (unblocks kv0),
        # then the remaining chunks interleaved
        w_sbs = {}

        def load_w(name, wdram):
            wsb = big.tile([P, CT, DCORE], BF16, tag=f"{name}_sb",
                           name=f"{name}_sb")
            nc.sync.dma_start(out=wsb, in_=wdram)
            w_sbs[name] = wsb

        nc.sync.dma_start_transpose(xT[0], x[0:TCH, :])
        load_w("wq", wq)
        nc.sync.dma_start_transpose(xeT[0], xe[0:TCH, :])
        load_w("wk", wk)
        load_w("wv", wv)
        for c in range(1, NCH):
            nc.sync.dma_start_transpose(xT[c], x[c * TCH:(c + 1) * TCH, :])
            nc.sync.dma_start_transpose(xeT[c], xe[c * TCH:(c + 1) * TCH, :])

        vtc = ctx.enter_context(tc.tile_pool(name="vtc", bufs=2))
        # projection psums share the score pool's rotation: psum has only
        # 8 banks and scores + pv accumulators need all of them
        sps = ctx.enter_context(tc.tile_pool(name="sps", bufs=6,
                                             space="PSUM"))
        pps = sps
        pvps = ctx.enter_context(tc.tile_pool(name="pvps", bufs=2,
                                              space="PSUM"))
        psb = ctx.enter_context(tc.tile_pool(name="psb", bufs=8))
        osb = ctx.enter_context(tc.tile_pool(name="osb", bufs=4))

        def proj(ps, wname, pair, src, c):
            w = w_sbs[wname]
            for ct in range(CT):
                nc.tensor.matmul(
                    ps, w[:, ct, pair * P:(pair + 1) * P], src[c][:, ct, :],
                    start=(ct == 0), stop=(ct == CT - 1))

        def emit_qt(c):
            csl = slice(c * TCH, (c + 1) * TCH)
            for pair in range(NPAIR):
                ps = pps.tile([P, TCH], F32, tag="s", name="qps")
                proj(ps, "wq", pair, xT, c)
                if pair == 0:
                    nc.scalar.copy(out=qT[:, pair, csl], in_=ps)
                else:
                    nc.vector.tensor_copy(out=qT[:, pair, csl], in_=ps)

        def emit_kv(c):
            csl = slice(c * TCH, (c + 1) * TCH)
            for pair in range(NPAIR):
                ps = pps.tile([P, TCH], F32, tag="s", name="kps")
                proj(ps, "wk", pair, xeT, c)
                if pair == 0:
                    nc.scalar.copy(out=kT[:, pair, csl], in_=ps)
                else:
                    nc.vector.tensor_copy(out=kT[:, pair, csl], in_=ps)
            for pair in range(NPAIR):
                ps = pps.tile([P, TCH], F32, tag="s", name="vps")
                proj(ps, "wv", pair, xeT, c)
                vt = vtc.tile([P, TCH], BF16, tag="vt", name="vt")
                nc.vector.tensor_copy(out=vt, in_=ps)
                for h2 in range(2):
                    # [d, s-chunk] -> contiguous [s%128, st, d], then a
                    # DVE copy into v_sb's 65-strided head slot
                    vtr = vtc.tile([P, 4, D], BF16, tag="vtr", name="vtr")
                    nc.sync.dma_start_transpose(
                        vtr, vt[h2 * D:(h2 + 1) * D, :])
                    nc.vector.tensor_copy(
                        out=v_sb[:, c * 4:(c + 1) * 4, 2 * pair + h2, 0:D],
                        in_=vtr)

        # Attention: software-pipelined loop over (group, s-tile-pair)
        # units, with GROUPS PROCESSED IN INTERLEAVED PAIRS: units of two
        # head-groups alternate, doubling the PE work between any score->
        # exp->pv dependency chain so the in-order PE never stalls on the
        # ~0.6us exp instructions. Scores are emitted LOOK entries ahead.
        # The kv/qT projections for chunks 1-3 are spliced between the
        # first pair's units: the PE chews on them while later xe chunks
        # stream in, and the exp engines warm up before the steady state.
        NSP = ST // 2
        groups = [(tch, pair, h2)
                  for tch in range(NTCH)
                  for pair in range(NPAIR)
                  for h2 in range(2)]
        NG = len(groups)
        entries = [(g, sp) for g in range(NG) for sp in range(NSP)]
        s_tiles = {}
        pv_tiles = {}
        # kv chunk c feeds s-tiles 4c..4c+3 = sp pairs 2c..2c+1 of the
        # first group pair; qT chunks feed later tch blocks
        hooks = {1: [lambda: emit_kv(1)],
                 3: [lambda: emit_kv(2), lambda: emit_qt(1)],
                 5: [lambda: emit_kv(3), lambda: emit_qt(2)],
                 7: [lambda: emit_qt(3)]}

        def emit_score_half(e, j):
            # each half-entry (one s-tile) gets its own single-bank psum
            # tile: six tiles rotate, so the score->exp->pv->reuse round
            # trip never throttles the PE
            g, sp = e
            tch, pair, h2 = groups[g]
            tsl = slice(tch * TCH, (tch + 1) * TCH)
            s_ps = sps.tile([P, TCH], F32, tag="s", name="s_ps")
            s_tiles[(e, j)] = s_ps
            st = 2 * sp + j
            nc.tensor.matmul(
                s_ps,
                kT[h2 * D:(h2 + 1) * D, pair, st * P:(st + 1) * P],
                qT[h2 * D:(h2 + 1) * D, pair, tsl],
                start=True, stop=True,
                tile_position=(h2 * D, 0))

        def emit_pv_alloc(g):
            # four 65-wide accumulators packed in one (bank-aligned) psum
            # bank; the group's very first matmul runs start=True, whose
            # 2KB zero-region pending-zero covers all four chains, so no
            # memset is needed and the other chains accumulate with
            # start=False
            pv_tiles[g] = pvps.tile([P, 4, D + 1], F32, tag="pv", name="pv")

        def emit_drain(g):
            gtch, gpair, gh2 = groups[g]
            o_t = osb.tile([P, 4, D + 1], F32, tag="o", name="o_t")
            nc.scalar.copy(out=o_t, in_=pv_tiles.pop(g))
            # SWDGE keeps stores off the SP queue feeding loads
            nc.gpsimd.dma_start(out=o[gtch, 2 * gpair + gh2], in_=o_t)

        emit_qt(0)
        emit_kv(0)
        emit_pv_alloc(0)
        for e in entries[:LOOK]:
            emit_score_half(e, 0)
            emit_score_half(e, 1)
        for i, e in enumerate(entries):
            g, sp = e
            tch, pair, h2 = groups[g]
            h = 2 * pair + h2
            if g < 2 and i in hooks:
                for fn in hooks[i]:
                    fn()
            if i + LOOK < len(entries):
                emit_score_half(entries[i + LOOK], 0)
                emit_score_half(entries[i + LOOK], 1)
            # each entry's exp is split by s-tile half: even s-tiles get
            # exact exp on ScalarE, odd s-tiles the calibrated Schraudolph
            # on DVE; the halves run concurrently on both engines
            pv = pv_tiles[g]
            for j in range(2):
                st = 2 * sp + j
                s_ps = s_tiles.pop((e, j))
                p_t = psb.tile([P, TCH], BF16, tag="p", name="p_t")
                if j == 0:
                    nc.scalar.activation(out=p_t, in_=s_ps, func=AF.Exp,
                                         scale=SCALE)
                else:
                    nc.vector.tensor_scalar(
                        out=p_t.bitcast(I16), in0=s_ps,
                        scalar1=SCH_A, scalar2=SCH_B,
                        op0=mybir.AluOpType.mult, op1=mybir.AluOpType.add)
                for tb in range(4):
                    nc.tensor.matmul(
                        pv[:, tb, :],
                        p_t[:, tb * P:(tb + 1) * P],
                        v_sb[:, st, h, :],
                        start=(sp == 0 and j == 0 and tb == 0),
                        stop=(st == ST - 1),
                        skip_group_check=True)
            if sp == 6:
                # slack window late in each group: retire the previous
                # group and allocate the next one's bank
                if g >= 1:
                    emit_drain(g - 1)
                if g + 1 < NG:
                    emit_pv_alloc(g + 1)
        emit_drain(NG - 1)


def build_program():
    nc = bacc.Bacc("TRN2", target_bir_lowering=False, debug=False,
                   num_devices=N_CORES)

    x = nc.dram_tensor("x", [T, C], BF16, kind="ExternalInput").ap()
    xe = nc.dram_tensor("xe", [S, C], BF16, kind="ExternalInput").ap()
    wq = nc.dram_tensor("wq", [P, CT, DCORE], BF16, kind="ExternalInput").ap()
    wk = nc.dram_tensor("wk", [P, CT, DCORE], BF16, kind="ExternalInput").ap()
    wv = nc.dram_tensor("wv", [P, CT, DCORE], BF16, kind="ExternalInput").ap()
    # per (tch, h): [t%512 partition, t-block, d+1]; col D = softmax denom
    o = nc.dram_tensor("o", [NTCH, HG, P, 4, D + 1], F32,
                       kind="ExternalOutput").ap()

    with tile.TileContext(nc) as tc:
        _build_body(nc, tc, x, xe, wq, wk, wv, o)
    nc.compile()
    return nc


_NC_CACHE = None


def _get_program():
    global _NC_CACHE
    if _NC_CACHE is None:
        _NC_CACHE = build_program()
    return _NC_CACHE


def _stage_w(wfull, csl):
    # [1024, 256] slice -> [P, CT, DCORE] bf16 with w[p, ct, d] = W[ct*128+p, d]
    wslc = np.ascontiguousarray(wfull[:, csl]).astype(ml_dtypes.bfloat16)
    return np.ascontiguousarray(wslc.reshape(CT, P, DCORE).transpose(1, 0, 2))


def kernel(x_enc, x, Wk, Wq, Wv):
    x_enc = np.asarray(x_enc, dtype=np.float32)
    x = np.asarray(x, dtype=np.float32)
    Wk = np.asarray(Wk, dtype=np.float32)
    Wq = np.asarray(Wq, dtype=np.float32)
    Wv = np.asarray(Wv, dtype=np.float32)

    nc = _get_program()
    in_maps = []
    for core in range(N_CORES):
        b, hg = divmod(core, N_CORES // B)
        csl = slice(hg * DCORE, (hg + 1) * DCORE)
        in_maps.append({
            "x": np.ascontiguousarray(x[b]).astype(ml_dtypes.bfloat16),
            "xe": np.ascontiguousarray(x_enc[b]).astype(ml_dtypes.bfloat16),
            "wq": _stage_w(Wq, csl),
            "wk": _stage_w(Wk, csl),
            "wv": _stage_w(Wv, csl),
        })
    res = run_bass_kernel_spmd(nc, in_maps, list(range(N_CORES)))

    full = np.empty((B, T, H, D), dtype=np.float32)
    for core in range(N_CORES):
        b, hg = divmod(core, N_CORES // B)
        o = res.results[core]["o"]          # [NTCH, HG, P, 4, D+1] f32
        num = o[..., :D]
        den = o[..., D]
        out = num / den[..., None]          # [tch, h, p, tb, d]
        out = out.transpose(0, 3, 2, 1, 4).reshape(T, HG, D)
        full[b, :, hg * HG:(hg + 1) * HG, :] = out
    return full
